# revision 1
# baseline (speedup 1.0000x reference)
"""Cross-attention kernel for Trainium2, SPMD across 8 NeuronCores.

Problem shapes (hardcoded): x [4, 2048, 512], mlp_out [4, 2048, 512],
Wq/Wk/Wv/Wp [512, 512], biases [512]. 8 heads x 64 head-dim.

Sharding: core c handles batch b = c//2 and query rows
[(c%2)*1024 : (c%2+1)*1024).  K/V work is duplicated across the two
cores of a batch pair; in exchange no collective is needed.

Design notes (v2):
  - The scalar engine's exp stream (128 x [128,1024] ~ 133us) is the
    hard floor; everything else is shaped to fit underneath it.
  - logits are computed transposed ([k, q]) per head with the head
    PAIR row-tiled on the PE array (contraction=64 at array rows
    0:64 / 64:128, concurrent), so a pair's two [128,1024] logit
    tiles cost ~the cycles of one.
  - AV makes V the stationary operand (64-col LDWEIGHTS instead of
    one 128-col load per tiny matmul) and the bf16 attn tile the
    moving operand; the two heads of a pair col-tile the array
    (outputs at PSUM partitions 0:64 / 64:128, concurrent).  The AV
    output is [dh, q] - already transposed for the output
    projection, so no PE transposes at all.
  - softmax denominators come from ones-vector matmuls col-tiled
    4-way (partitions 0/32/64/96 of one PSUM bank); the reciprocal
    row is broadcast across 64 partitions by a rank-1 matmul and
    applied as one [128,512] DVE multiply during the AV eviction.
  - attention is software-pipelined per (pair, kt): exp for step s
    is emitted before AV for step s-1, so the scalar engine never
    waits on the PE and attn tiles free after one step.
  - PSUM is exactly 8 banks: logits 2x[128,1024] (4) + AV [128,1024]
    (2) + denominators [128,512] (1) + one projection bank.  The
    projection bank is single-buffered; its eviction latency hides
    behind interleaved attention matmuls, and the mt>=2 Q/K
    projection groups are deferred into the pair-1 attention stream
    (their results aren't needed until pairs 2/3).
"""

import os

import numpy as np

import concourse.bass as bass
import concourse.tile as tile
from concourse import bacc, mybir
from concourse.bass_utils import run_bass_kernel_spmd

# timing-only ablation knobs (default off; results are WRONG when set)
_SKIP_DEN = os.environ.get("K_SKIP_DEN") == "1"
_SKIP_AVODD = os.environ.get("K_SKIP_AVODD") == "1"
_PRODUCE_ONLY = os.environ.get("K_PRODUCE_ONLY") == "1"
_NO_EXTRAS = os.environ.get("K_NO_EXTRAS") == "1"
_LP_BUFS = int(os.environ.get("K_LP_BUFS", "2"))

B = 4
N = 2048          # both query and key/value sequence length
C = 512           # model dim
H = 8
D = C // H        # 64
NCORES = 8
QSH = N // 2      # query rows per core (1024)

F32 = mybir.dt.float32
F32R = mybir.dt.float32r
BF16 = mybir.dt.bfloat16

P = 128
CT = C // P       # 4 tiles along any model-dim axis (also: head pairs)
QT = QSH // P     # 8 query tiles
KT = N // P       # 16 key tiles
QB = QSH // 512   # 2 query blocks of 512 (fp32-class moving-dim limit)
KB = N // 512     # 4


def build_nc(with_bias: bool, reps: int = 1):
    nc = bacc.Bacc("TRN2", target_bir_lowering=False, debug=False)

    xT = nc.dram_tensor("xT", [C, QSH], F32R, kind="ExternalInput")
    mlpT = nc.dram_tensor("mlpT", [C, N], F32R, kind="ExternalInput")
    wqT = nc.dram_tensor("wqT", [C, C], F32R, kind="ExternalInput")
    wkT = nc.dram_tensor("wkT", [C, C], F32R, kind="ExternalInput")
    wvT = nc.dram_tensor("wvT", [C, C], F32R, kind="ExternalInput")
    wpT = nc.dram_tensor("wpT", [C, C], F32R, kind="ExternalInput")
    if with_bias:
        bq = nc.dram_tensor("bq", [1, C], F32, kind="ExternalInput")
        bk = nc.dram_tensor("bk", [1, C], F32, kind="ExternalInput")
        bv = nc.dram_tensor("bv", [1, C], F32, kind="ExternalInput")
        bp = nc.dram_tensor("bp", [1, C], F32, kind="ExternalInput")
    out = nc.dram_tensor("out", [QSH, C], F32, kind="ExternalOutput")

    with tile.TileContext(nc) as tc:
        from contextlib import ExitStack

        with ExitStack() as ctx:
            const = ctx.enter_context(tc.tile_pool(name="const", bufs=1))
            wq_pool = ctx.enter_context(tc.tile_pool(name="wq", bufs=1))
            wk_pool = ctx.enter_context(tc.tile_pool(name="wk", bufs=1))
            wv_pool = ctx.enter_context(tc.tile_pool(name="wv", bufs=1))
            wp_pool = ctx.enter_context(tc.tile_pool(name="wp", bufs=1))
            xin_pool = ctx.enter_context(tc.tile_pool(name="xin", bufs=2))
            min_pool = ctx.enter_context(tc.tile_pool(name="min", bufs=4))
            qt_pool = ctx.enter_context(tc.tile_pool(name="qTp", bufs=1))
            kt_pool = ctx.enter_context(tc.tile_pool(name="kTp", bufs=1))
            v_pool = ctx.enter_context(tc.tile_pool(name="vp", bufs=1))
            aoT_pool = ctx.enter_context(tc.tile_pool(name="aoT", bufs=1))
            attn_pool = ctx.enter_context(tc.tile_pool(name="attnT", bufs=8))
            small = ctx.enter_context(tc.tile_pool(name="small", bufs=2))
            outst = ctx.enter_context(tc.tile_pool(name="outst", bufs=2))
            proj_ps = ctx.enter_context(
                tc.tile_pool(name="proj_ps", bufs=1, space="PSUM")
            )
            logits_ps = ctx.enter_context(
                tc.tile_pool(name="logits_ps", bufs=_LP_BUFS, space="PSUM")
            )
            av_ps = ctx.enter_context(
                tc.tile_pool(name="av_ps", bufs=1, space="PSUM")
            )
            den_ps = ctx.enter_context(
                tc.tile_pool(name="den_ps", bufs=1, space="PSUM")
            )

            # ---- constants ----
            onesr_f = const.tile([1, P], F32)
            nc.vector.memset(onesr_f[:], 1.0)
            onesr = const.tile([1, P], F32R)
            nc.vector.tensor_copy(onesr[:], onesr_f[:])
            ones128 = const.tile([P, 1], BF16)
            nc.vector.memset(ones128[:], 1.0)

            for _rep in range(reps):

              if with_bias:
                  def load_row_f32r(dram_row, nm):
                      f = const.tile([1, C], F32, name=f"{nm}_f")
                      nc.sync.dma_start(out=f[:], in_=dram_row[:])
                      r = const.tile([1, C], F32R, name=f"{nm}_r")
                      nc.vector.tensor_copy(r[:], f[:])
                      return r

                  bq_r = load_row_f32r(bq, "bq")
                  bk_r = load_row_f32r(bk, "bk")
                  bv_r = load_row_f32r(bv, "bv")
                  bp_r = load_row_f32r(bp, "bp")
                  ones_f = const.tile([1, 512], F32)
                  nc.vector.memset(ones_f[:], 1.0)
                  ones_r = const.tile([1, 512], F32R)
                  nc.vector.tensor_copy(ones_r[:], ones_f[:])
                  onec = const.tile([1, P], F32R)
                  nc.vector.tensor_copy(onec[:], ones_r[:, 0:P])

              # ---- input DMAs.  Weight tiles are [128, 4, 512]: wq[p,cc,n]
              # = wqT[cc*128+p, n].  wq/wk arrive split so the mt=0 slice
              # (all pair-0 needs) lands as early as possible. ----
              def w_tile(pool, nm):
                  return pool.tile([P, CT, C], F32R, tag=nm, name=nm)

              def dma_w(t, dram, n0, n1):
                  nc.sync.dma_start(
                      out=t[:, :, n0:n1],
                      in_=dram[:, n0:n1].rearrange("(c p) n -> p c n", p=P),
                  )

              wq = w_tile(wq_pool, "wq")
              dma_w(wq, wqT, 0, P)
              xblks = []
              for qb in range(QB):
                  t = xin_pool.tile([P, CT, 512], F32R, tag="xin",
                                    name=f"xb{qb}")
                  nc.sync.dma_start(
                      out=t[:],
                      in_=xT[:, qb * 512:(qb + 1) * 512].rearrange(
                          "(c p) n -> p c n", p=P),
                  )
                  xblks.append(t)
              wk = w_tile(wk_pool, "wk")
              dma_w(wk, wkT, 0, P)
              mblks = []
              for kb in range(KB):
                  t = min_pool.tile([P, CT, 512], F32R, tag="min",
                                    name=f"mb{kb}")
                  if kb == 0:
                      nc.sync.dma_start(
                          out=t[:],
                          in_=mlpT[:, 0:512].rearrange("(c p) n -> p c n",
                                                       p=P),
                      )
                  mblks.append(t)
              wv = w_tile(wv_pool, "wv")
              dma_w(wv, wvT, 0, C)
              dma_w(wq, wqT, P, C)
              dma_w(wk, wkT, P, C)
              for kb in range(1, KB):
                  nc.sync.dma_start(
                      out=mblks[kb][:],
                      in_=mlpT[:, kb * 512:(kb + 1) * 512].rearrange(
                          "(c p) n -> p c n", p=P),
                  )
              wp = w_tile(wp_pool, "wp")
              dma_w(wp, wpT, 0, C)

              qT = [qt_pool.tile([P, QSH], F32R, tag=f"qT{i}", name=f"qT{i}")
                    for i in range(CT)]
              kT = [kt_pool.tile([P, N], F32R, tag=f"kT{i}", name=f"kT{i}")
                    for i in range(CT)]
              # v tiles: [k-part, dh] per kt; lhsT slices are 64-col head dims
              vt = [v_pool.tile([P, C], BF16, tag=f"v{i}", name=f"v{i}")
                    for i in range(KT)]
              # attn-out transposed: per pair g, [128 dh, 1024 q]
              aoT = [aoT_pool.tile([P, QSH], F32R, tag=f"aoT{g}",
                                   name=f"aoT{g}") for g in range(CT)]

              def proj_qT(mt, qb):
                  ps = proj_ps.tile([P, 512], F32, tag="proj", name="ps_q")
                  for cc in range(CT):
                      nc.tensor.matmul(
                          ps[:],
                          wq[:, cc, mt * P:(mt + 1) * P],
                          xblks[qb][:, cc, :],
                          start=(cc == 0),
                          stop=(cc == CT - 1 and not with_bias),
                      )
                  if with_bias:
                      nc.tensor.matmul(
                          ps[:], bq_r[:, mt * P:(mt + 1) * P], ones_r[:],
                          start=False, stop=True,
                      )
                  nc.vector.tensor_copy(qT[mt][:, qb * 512:(qb + 1) * 512],
                                        ps[:])

              def proj_kT(mt, kb):
                  ps = proj_ps.tile([P, 512], F32, tag="proj", name="ps_k")
                  for cc in range(CT):
                      nc.tensor.matmul(
                          ps[:],
                          wk[:, cc, mt * P:(mt + 1) * P],
                          mblks[kb][:, cc, :],
                          start=(cc == 0),
                          stop=(cc == CT - 1 and not with_bias),
                      )
                  if with_bias:
                      nc.tensor.matmul(
                          ps[:], bk_r[:, mt * P:(mt + 1) * P], ones_r[:],
                          start=False, stop=True,
                      )
                  nc.vector.tensor_copy(kT[mt][:, kb * 512:(kb + 1) * 512],
                                        ps[:])

              def proj_v(kt):
                  kb, lo = kt // 4, (kt % 4) * P
                  ps = proj_ps.tile([P, 512], F32, tag="proj", name="ps_v")
                  for cc in range(CT):
                      nc.tensor.matmul(
                          ps[:],
                          mblks[kb][:, cc, lo:lo + P],
                          wv[:, cc, :],
                          start=(cc == 0),
                          stop=(cc == CT - 1 and not with_bias),
                      )
                  if with_bias:
                      nc.tensor.matmul(ps[:], onec[:], bv_r[:],
                                       start=False, stop=True)
                  nc.vector.tensor_copy(vt[kt][:], ps[:])

              # ---- attention pipeline ----
              cur_av = [None]    # [128,1024] psum: 0:64 even / 64:128 odd
              cur_den = [None]   # [128,512] psum: denoms at 0/32/64/96
              attn_tiles = {}    # (h, kt) -> sbuf tile [128, 1024] bf16

              def produce(g, kt):
                  """logits (row-tiled pair) + exp for step (g, kt)."""
                  for hh in range(2):
                      lp = logits_ps.tile([P, QSH], F32, tag="lp", name="lp")
                      po = hh * D
                      for qb in range(QB):
                          nc.tensor.matmul(
                              lp[:, qb * 512:(qb + 1) * 512],
                              kT[g][po:po + D, kt * P:(kt + 1) * P],
                              qT[g][po:po + D, qb * 512:(qb + 1) * 512],
                              start=True,
                              stop=True,
                          )
                      at = attn_pool.tile([P, QSH], BF16, tag="attnT",
                                          name="at")
                      nc.scalar.activation(
                          out=at[:], in_=lp[:],
                          func=mybir.ActivationFunctionType.Exp,
                      )
                      attn_tiles[(2 * g + hh, kt)] = at

              def consume(g, kt):
                  """AV (col-tiled pair) + denominator matmuls."""
                  if kt == 0:
                      cur_av[0] = av_ps.tile([P, QSH], F32, tag="av",
                                             name="av")
                      cur_den[0] = den_ps.tile([P, 512], F32, tag="den",
                                               name="den")
                  av, den = cur_av[0], cur_den[0]
                  a_e = attn_tiles.pop((2 * g, kt))
                  a_o = attn_tiles.pop((2 * g + 1, kt))
                  st, sp = (kt == 0), (kt == KT - 1)
                  # even-tile readers first, then odd: by consume time both
                  # attn tiles are old (LAG 2), but keep any residual waits
                  # clustered so the wait queue never blocks dispatch
                  for qb in range(QB):
                      qs = slice(qb * 512, (qb + 1) * 512)
                      nc.tensor.matmul(
                          av[0:D, qs], vt[kt][:, g * P:g * P + D],
                          a_e[:, qs], start=st, stop=sp,
                      )
                  for qb in range(QB):
                      if _SKIP_DEN and not st:
                          break
                      p0 = 32 * qb
                      nc.tensor.matmul(
                          den[p0:p0 + 1, :], ones128[:],
                          a_e[:, qb * 512:(qb + 1) * 512],
                          start=st, stop=(sp or _SKIP_DEN),
                          tile_position=(0, p0),
                      )
                  for qb in range(QB):
                      qs = slice(qb * 512, (qb + 1) * 512)
                      nc.tensor.matmul(
                          av[D:2 * D, qs],
                          vt[kt][:, g * P + D:(g + 1) * P],
                          a_o[:, qs], start=st, stop=sp,
                      )
                  for qb in range(QB):
                      if _SKIP_DEN and not st:
                          break
                      p0 = 64 + 32 * qb
                      nc.tensor.matmul(
                          den[p0:p0 + 1, :], ones128[:],
                          a_o[:, qb * 512:(qb + 1) * 512],
                          start=st, stop=(sp or _SKIP_DEN),
                          tile_position=(0, p0),
                      )

              def normalize(g):
                  """reciprocal + broadcast + apply; writes aoT[g].
                  The K=1 broadcast matmul must target col position 0 with
                  M=128 (M=64 at position 64 fails the ISA dst check), so
                  each (head, qb) gets a full-height broadcast and the
                  multiply slices the matching 64 partitions."""
                  av, den = cur_av[0], cur_den[0]
                  for qb in range(QB):
                      for j in range(2):
                          p0 = 64 * j + 32 * qb
                          rf = small.tile([1, 512], F32, tag="rf", name="rf")
                          nc.vector.reciprocal(rf[:], den[p0:p0 + 1, :])
                          rr = small.tile([1, 512], F32R, tag="rr", name="rr")
                          nc.vector.tensor_copy(rr[:], rf[:])
                          bc = proj_ps.tile([P, 512], F32, tag="proj",
                                            name="bc")
                          nc.tensor.matmul(bc[:], onesr[:], rr[:],
                                           start=True, stop=True)
                          bs = small.tile([P, 512], F32R, tag="bs", name="bs")
                          nc.vector.tensor_copy(bs[:], bc[:])
                          hs = slice(64 * j, 64 * (j + 1))
                          qs = slice(qb * 512, (qb + 1) * 512)
                          nc.vector.tensor_tensor(
                              out=aoT[g][hs, qs],
                              in0=av[hs, qs],
                              in1=bs[hs, :],
                              op=mybir.AluOpType.mult,
                          )

              def outproj(qt):
                  # alternate PSUM banks (proj / recycled av) so the
                  # matmul group of qt+1 overlaps the eviction of qt
                  if qt % 2 == 0:
                      po = proj_ps.tile([P, 512], F32, tag="proj",
                                        name="ps_o")
                  else:
                      po = av_ps.tile([P, 512], F32, tag="av", name="ps_o2")
                  for g in range(CT):
                      nc.tensor.matmul(
                          po[:],
                          aoT[g][:, qt * P:(qt + 1) * P],
                          wp[:, g, :],
                          start=(g == 0),
                          stop=(g == CT - 1 and not with_bias),
                      )
                  if with_bias:
                      nc.tensor.matmul(po[:], onec[:], bp_r[:],
                                       start=False, stop=True)
                  o = outst.tile([P, C], F32, tag="outst", name="outst")
                  nc.vector.tensor_copy(o[:], po[:])
                  nc.sync.dma_start(out=out[qt * P:(qt + 1) * P, :], in_=o[:])

              # ---- unified 64-step pipeline: produce(s) then
              # consume(s-1); normalize(g) inline after consume(g, 15);
              # projection groups ride as per-step extras ----
              extras = {s: [] for s in range(64)}

              def put(s, fn, *a):
                  extras[s].append((fn, a))

              # pair 0 window: V projections land just-in-time; kT[0]
              # chunks prefetch one kb ahead; mt1 projections by step 16
              for kt in range(1, KT):
                  put(kt - 1, proj_v, kt)
              put(0, proj_kT, 0, 1)
              put(3, proj_kT, 0, 2)
              put(6, proj_kT, 0, 3)
              put(1, proj_qT, 1, 0)
              put(4, proj_qT, 1, 1)
              for kb in range(KB):
                  put(8 + 2 * kb, proj_kT, 1, kb)
              # pair 1 window: mt2 by step 32, mt3 by step 48
              put(16, proj_qT, 2, 0)
              put(18, proj_qT, 2, 1)
              for kb in range(KB):
                  put(20 + 2 * kb, proj_kT, 2, kb)
              put(28, proj_qT, 3, 0)
                  # mt3 K-projections stretch into the pair-2 window
              put(30, proj_qT, 3, 1)
              for kb in range(KB):
                  put(32 + 2 * kb, proj_kT, 3, kb)

              proj_qT(0, 0)
              proj_qT(0, 1)
              proj_kT(0, 0)
              proj_v(0)
              if _NO_EXTRAS:
                  # diagnostic: run every projection upfront; the steady
                  # state then carries no extra PE work (slow ramp).
                  moved = [t for s in sorted(extras) for t in extras[s]]
                  extras = {s: [] for s in range(64)}
                  for fn, a in moved:
                      fn(*a)
              steps = [(g, kt) for g in range(CT) for kt in range(KT)]
              if _PRODUCE_ONLY:
                  # cadence probe: logits+exp stream only (WRONG results).
                  for kb in range(1, KB):
                      proj_kT(0, kb)
                  for s in range(len(steps)):
                      produce(0, s % KT)
                      attn_tiles.clear()
                  for g in range(CT):
                      nc.vector.tensor_copy(aoT[g][:], qT[0][:])
              else:
                  LAG = 2  # consume deps are 2 steps old: no PE waits
                  for s, (g, kt) in enumerate(steps):
                      produce(g, kt)
                      for fn, a in extras[s]:
                          fn(*a)
                      if s >= LAG:
                          pg, pkt = steps[s - LAG]
                          consume(pg, pkt)
                          if pkt == KT - 1:
                              normalize(pg)
                  for s in range(len(steps) - LAG, len(steps)):
                      pg, pkt = steps[s]
                      consume(pg, pkt)
                      if pkt == KT - 1:
                          normalize(pg)

              # ---- output projection tail ----
              for qt in range(QT):
                  outproj(qt)

    nc.compile()
    return nc


_CACHE: dict = {}


def get_nc(with_bias: bool):
    key = ("nc", with_bias)
    if key not in _CACHE:
        _CACHE[key] = build_nc(with_bias)
    return _CACHE[key]


def make_in_maps(inputs: dict) -> tuple[list[dict], bool]:
    x = np.asarray(inputs["x"], dtype=np.float32)
    mlp = np.asarray(inputs["mlp_out"], dtype=np.float32)
    Wq = np.asarray(inputs["Wq"], dtype=np.float32)
    Wk = np.asarray(inputs["Wk"], dtype=np.float32)
    Wv = np.asarray(inputs["Wv"], dtype=np.float32)
    Wp = np.asarray(inputs["Wp"], dtype=np.float32)
    bq = np.asarray(inputs["bq"], dtype=np.float32)
    bk = np.asarray(inputs["bk"], dtype=np.float32)
    bv = np.asarray(inputs["bv"], dtype=np.float32)
    bp = np.asarray(inputs["bp"], dtype=np.float32)

    with_bias = bool(np.any(bq) or np.any(bk) or np.any(bv) or np.any(bp))

    wqT = np.ascontiguousarray(Wq.T)  # [c, dh]
    wkT = np.ascontiguousarray(Wk.T)
    wvT = np.ascontiguousarray(Wv.T)
    wpT = np.ascontiguousarray(Wp.T)  # [dh, co]

    in_maps = []
    for c in range(NCORES):
        b, half = c // 2, c % 2
        xs = np.ascontiguousarray(x[b, half * QSH:(half + 1) * QSH, :].T)
        ms = np.ascontiguousarray(mlp[b].T)
        m = {
            "xT": xs, "mlpT": ms,
            "wqT": wqT, "wkT": wkT, "wvT": wvT, "wpT": wpT,
        }
        if with_bias:
            m["bq"] = bq.reshape(1, C)
            m["bk"] = bk.reshape(1, C)
            m["bv"] = bv.reshape(1, C)
            m["bp"] = bp.reshape(1, C)
        in_maps.append(m)
    return in_maps, with_bias


def kernel(**inputs) -> np.ndarray:
    in_maps, with_bias = make_in_maps(inputs)
    nc = get_nc(with_bias)
    res = run_bass_kernel_spmd(nc, in_maps, list(range(NCORES)))
    full = np.empty((B, N, C), dtype=np.float32)
    for c in range(NCORES):
        b, half = c // 2, c % 2
        full[b, half * QSH:(half + 1) * QSH, :] = res.results[c]["out"]
    return full



# revision 19
# speedup vs baseline: 1.3793x; 1.3793x over previous
"""Cross-attention kernel for Trainium2, SPMD across 8 NeuronCores.

Problem shapes (hardcoded): x [4, 2048, 512], mlp_out [4, 2048, 512],
Wq/Wk/Wv/Wp [512, 512], biases [512]. 8 heads x 64 head-dim.

Sharding: core c handles batch b = c//2 and query rows
[(c%2)*1024 : (c%2+1)*1024).  K/V work is duplicated across the two
cores of a batch pair; in exchange no collective is needed.

Design notes (v2):
  - The scalar engine's exp stream (128 x [128,1024] ~ 133us) is the
    hard floor; everything else is shaped to fit underneath it.
  - logits are computed transposed ([k, q]) per head with the head
    PAIR row-tiled on the PE array (contraction=64 at array rows
    0:64 / 64:128, concurrent), so a pair's two [128,1024] logit
    tiles cost ~the cycles of one.
  - AV makes V the stationary operand (64-col LDWEIGHTS instead of
    one 128-col load per tiny matmul) and the bf16 attn tile the
    moving operand; the two heads of a pair col-tile the array
    (outputs at PSUM partitions 0:64 / 64:128, concurrent).  The AV
    output is [dh, q] - already transposed for the output
    projection, so no PE transposes at all.
  - softmax denominators come from ones-vector matmuls col-tiled
    4-way (partitions 0/32/64/96 of one PSUM bank); the reciprocal
    row is broadcast across 64 partitions by a rank-1 matmul and
    applied as one [128,512] DVE multiply during the AV eviction.
  - attention is software-pipelined per (pair, kt): exp for step s
    is emitted before AV for step s-1, so the scalar engine never
    waits on the PE and attn tiles free after one step.
  - PSUM is exactly 8 banks: logits 2x[128,1024] (4) + AV [128,1024]
    (2) + denominators [128,512] (1) + one projection bank.  The
    projection bank is single-buffered; its eviction latency hides
    behind interleaved attention matmuls, and the mt>=2 Q/K
    projection groups are deferred into the pair-1 attention stream
    (their results aren't needed until pairs 2/3).
"""

import os

import numpy as np

import concourse.bass as bass
import concourse.tile as tile
from concourse import bacc, mybir
from concourse.bass_utils import run_bass_kernel_spmd

# timing-only ablation knobs (default off; results are WRONG when set)
_SKIP_DEN = os.environ.get("K_SKIP_DEN") == "1"
_SKIP_AVODD = os.environ.get("K_SKIP_AVODD") == "1"
_PRODUCE_ONLY = os.environ.get("K_PRODUCE_ONLY") == "1"
_NO_EXTRAS = os.environ.get("K_NO_EXTRAS") == "1"
_LP_BUFS = int(os.environ.get("K_LP_BUFS", "2"))
# number of the 128 exp tiles offloaded to the DVE via the Schraudolph
# bit-trick (one tensor_scalar: i32 = round(x * 2^23/ln2 + B); the i32
# bit pattern IS the fp32 approximation of exp(x)).  ~3% sawtooth error
# per weight; cancels to ~0.8% in the final output at this share.
_N_DVE_EXP = int(os.environ.get("K_DVE_EXP", "44"))

B = 4
N = 2048          # both query and key/value sequence length
C = 512           # model dim
H = 8
D = C // H        # 64
NCORES = 8
QSH = N // 2      # query rows per core (1024)

F32 = mybir.dt.float32
F32R = mybir.dt.float32r
BF16 = mybir.dt.bfloat16
I16 = mybir.dt.int16

import math
# Schraudolph bit-exp in bf16-bit domain: i16 = round(x * 2^7/ln2 +
# (127*2^7 - C)); the int16 bit pattern IS bf16(exp(x)) up to a ~3%
# sawtooth (C centers it).  The constant offset cancels in softmax.
A_EXP = float(np.float32(2.0**7 / math.log(2.0)))
B_EXP = float(np.float32(127.0 * 2.0**7 - 5.5))

P = 128
CT = C // P       # 4 tiles along any model-dim axis (also: head pairs)
QT = QSH // P     # 8 query tiles
KT = N // P       # 16 key tiles
QB = QSH // 512   # 2 query blocks of 512 (fp32-class moving-dim limit)
KB = N // 512     # 4


def build_nc(with_bias: bool, reps: int = 1):
    nc = bacc.Bacc("TRN2", target_bir_lowering=False, debug=False)

    xT = nc.dram_tensor("xT", [C, QSH], F32R, kind="ExternalInput")
    mlpT = nc.dram_tensor("mlpT", [C, N], F32R, kind="ExternalInput")
    wqT = nc.dram_tensor("wqT", [C, C], F32R, kind="ExternalInput")
    wkT = nc.dram_tensor("wkT", [C, C], F32R, kind="ExternalInput")
    wvT = nc.dram_tensor("wvT", [C, C], F32R, kind="ExternalInput")
    wpT = nc.dram_tensor("wpT", [C, C], F32R, kind="ExternalInput")
    if with_bias:
        bq = nc.dram_tensor("bq", [1, C], F32, kind="ExternalInput")
        bk = nc.dram_tensor("bk", [1, C], F32, kind="ExternalInput")
        bv = nc.dram_tensor("bv", [1, C], F32, kind="ExternalInput")
        bp = nc.dram_tensor("bp", [1, C], F32, kind="ExternalInput")
    out = nc.dram_tensor("out", [QSH, C], F32, kind="ExternalOutput")

    with tile.TileContext(nc) as tc:
        from contextlib import ExitStack

        with ExitStack() as ctx:
            const = ctx.enter_context(tc.tile_pool(name="const", bufs=1))
            wq_pool = ctx.enter_context(tc.tile_pool(name="wq", bufs=1))
            wk_pool = ctx.enter_context(tc.tile_pool(name="wk", bufs=1))
            wv_pool = ctx.enter_context(tc.tile_pool(name="wv", bufs=1))
            wp_pool = ctx.enter_context(tc.tile_pool(name="wp", bufs=1))
            xin_pool = ctx.enter_context(tc.tile_pool(name="xin", bufs=2))
            min_pool = ctx.enter_context(tc.tile_pool(name="min", bufs=4))
            qt_pool = ctx.enter_context(tc.tile_pool(name="qTp", bufs=1))
            kt_pool = ctx.enter_context(tc.tile_pool(name="kTp", bufs=1))
            v_pool = ctx.enter_context(tc.tile_pool(name="vp", bufs=1))
            aoT_pool = ctx.enter_context(tc.tile_pool(name="aoT", bufs=1))
            attn_pool = ctx.enter_context(tc.tile_pool(name="attnT", bufs=8))
            small = ctx.enter_context(tc.tile_pool(name="small", bufs=2))
            outst = ctx.enter_context(tc.tile_pool(name="outst", bufs=2))
            proj_ps = ctx.enter_context(
                tc.tile_pool(name="proj_ps", bufs=1, space="PSUM")
            )
            logits_ps = ctx.enter_context(
                tc.tile_pool(name="logits_ps", bufs=_LP_BUFS, space="PSUM")
            )
            av_ps = ctx.enter_context(
                tc.tile_pool(name="av_ps", bufs=1, space="PSUM")
            )
            den_ps = ctx.enter_context(
                tc.tile_pool(name="den_ps", bufs=1, space="PSUM")
            )

            # ---- constants ----
            onesr_f = const.tile([1, P], F32)
            nc.vector.memset(onesr_f[:], 1.0)
            onesr = const.tile([1, P], F32R)
            nc.vector.tensor_copy(onesr[:], onesr_f[:])
            ones128 = const.tile([P, 1], BF16)
            nc.vector.memset(ones128[:], 1.0)
            # selection matrix for the normalize broadcast: one K=33
            # matmul maps recip rows {b, b+32} onto output partitions
            # 0:64 / 64:128 (b = 0 for qb0, 64 for qb1).
            sel_f = const.tile([P, P], F32)
            nc.vector.memset(sel_f[:], 0.0)
            for b in (0, 64):
                nc.vector.memset(sel_f[b:b + 1, 0:64], 1.0)
                nc.vector.memset(sel_f[b + 32:b + 33, 64:128], 1.0)
            sel = const.tile([P, P], F32R)
            nc.vector.tensor_copy(sel[:], sel_f[:])

            for _rep in range(reps):

              if with_bias:
                  def load_row_f32r(dram_row, nm):
                      f = const.tile([1, C], F32, name=f"{nm}_f")
                      nc.sync.dma_start(out=f[:], in_=dram_row[:])
                      r = const.tile([1, C], F32R, name=f"{nm}_r")
                      nc.vector.tensor_copy(r[:], f[:])
                      return r

                  bq_r = load_row_f32r(bq, "bq")
                  bk_r = load_row_f32r(bk, "bk")
                  bv_r = load_row_f32r(bv, "bv")
                  bp_r = load_row_f32r(bp, "bp")
                  ones_f = const.tile([1, 512], F32)
                  nc.vector.memset(ones_f[:], 1.0)
                  ones_r = const.tile([1, 512], F32R)
                  nc.vector.tensor_copy(ones_r[:], ones_f[:])
                  onec = const.tile([1, P], F32R)
                  nc.vector.tensor_copy(onec[:], ones_r[:, 0:P])

              # ---- input DMAs.  Weight tiles are [128, 4, 512]: wq[p,cc,n]
              # = wqT[cc*128+p, n].  wq/wk arrive split so the mt=0 slice
              # (all pair-0 needs) lands as early as possible. ----
              def w_tile(pool, nm):
                  return pool.tile([P, CT, C], F32R, tag=nm, name=nm)

              def dma_w(t, dram, n0, n1):
                  nc.sync.dma_start(
                      out=t[:, :, n0:n1],
                      in_=dram[:, n0:n1].rearrange("(c p) n -> p c n", p=P),
                  )

              wq = w_tile(wq_pool, "wq")
              dma_w(wq, wqT, 0, P)
              xblks = []
              for qb in range(QB):
                  t = xin_pool.tile([P, CT, 512], F32R, tag="xin",
                                    name=f"xb{qb}")
                  nc.sync.dma_start(
                      out=t[:],
                      in_=xT[:, qb * 512:(qb + 1) * 512].rearrange(
                          "(c p) n -> p c n", p=P),
                  )
                  xblks.append(t)
              wk = w_tile(wk_pool, "wk")
              dma_w(wk, wkT, 0, P)
              mblks = []
              for kb in range(KB):
                  t = min_pool.tile([P, CT, 512], F32R, tag="min",
                                    name=f"mb{kb}")
                  if kb == 0:
                      nc.sync.dma_start(
                          out=t[:],
                          in_=mlpT[:, 0:512].rearrange("(c p) n -> p c n",
                                                       p=P),
                      )
                  mblks.append(t)
              wv = w_tile(wv_pool, "wv")
              dma_w(wv, wvT, 0, C)
              dma_w(wq, wqT, P, C)
              dma_w(wk, wkT, P, C)
              for kb in range(1, KB):
                  nc.sync.dma_start(
                      out=mblks[kb][:],
                      in_=mlpT[:, kb * 512:(kb + 1) * 512].rearrange(
                          "(c p) n -> p c n", p=P),
                  )
              wp = w_tile(wp_pool, "wp")
              dma_w(wp, wpT, 0, C)

              qT = [qt_pool.tile([P, QSH], F32R, tag=f"qT{i}", name=f"qT{i}")
                    for i in range(CT)]
              kT = [kt_pool.tile([P, N], F32R, tag=f"kT{i}", name=f"kT{i}")
                    for i in range(CT)]
              # v tiles: [k-part, dh] per kt; lhsT slices are 64-col head dims
              vt = [v_pool.tile([P, C], BF16, tag=f"v{i}", name=f"v{i}")
                    for i in range(KT)]
              # attn-out transposed: per pair g, [128 dh, 1024 q]
              aoT = [aoT_pool.tile([P, QSH], F32R, tag=f"aoT{g}",
                                   name=f"aoT{g}") for g in range(CT)]

              def proj_qT(mt, qb):
                  ps = proj_ps.tile([P, 512], F32, tag="proj", name="ps_q")
                  for cc in range(CT):
                      nc.tensor.matmul(
                          ps[:],
                          wq[:, cc, mt * P:(mt + 1) * P],
                          xblks[qb][:, cc, :],
                          start=(cc == 0),
                          stop=(cc == CT - 1 and not with_bias),
                      )
                  if with_bias:
                      nc.tensor.matmul(
                          ps[:], bq_r[:, mt * P:(mt + 1) * P], ones_r[:],
                          start=False, stop=True,
                      )
                  nc.vector.tensor_copy(qT[mt][:, qb * 512:(qb + 1) * 512],
                                        ps[:])

              def proj_kT(mt, kb):
                  ps = proj_ps.tile([P, 512], F32, tag="proj", name="ps_k")
                  for cc in range(CT):
                      nc.tensor.matmul(
                          ps[:],
                          wk[:, cc, mt * P:(mt + 1) * P],
                          mblks[kb][:, cc, :],
                          start=(cc == 0),
                          stop=(cc == CT - 1 and not with_bias),
                      )
                  if with_bias:
                      nc.tensor.matmul(
                          ps[:], bk_r[:, mt * P:(mt + 1) * P], ones_r[:],
                          start=False, stop=True,
                      )
                  nc.vector.tensor_copy(kT[mt][:, kb * 512:(kb + 1) * 512],
                                        ps[:])

              def proj_v(kt):
                  kb, lo = kt // 4, (kt % 4) * P
                  ps = proj_ps.tile([P, 512], F32, tag="proj", name="ps_v")
                  for cc in range(CT):
                      nc.tensor.matmul(
                          ps[:],
                          mblks[kb][:, cc, lo:lo + P],
                          wv[:, cc, :],
                          start=(cc == 0),
                          stop=(cc == CT - 1 and not with_bias),
                      )
                  if with_bias:
                      nc.tensor.matmul(ps[:], onec[:], bv_r[:],
                                       start=False, stop=True)
                  nc.vector.tensor_copy(vt[kt][:], ps[:])

              # ---- attention pipeline ----
              cur_av = [None]    # [128,1024] psum: 0:64 even / 64:128 odd
              cur_den = [None]   # [128,512] psum: denoms at 0/32/64/96
              attn_tiles = {}    # (h, kt) -> sbuf tile [128, 1024]

              # Bresenham spread of the DVE-exp slots over the 128
              # (g, kt, hh) tiles so scalar/DVE interleave evenly.
              def use_dve(slot):
                  n = _N_DVE_EXP
                  return ((slot + 1) * n) // 128 > (slot * n) // 128

              def produce(g, kt):
                  """logits (row-tiled pair) + exp for step (g, kt).

                  The two heads' logits matmuls alternate (e_qb0, o_qb0,
                  e_qb1, o_qb1): the pair is row-tiled on the PE (rows
                  0:64 / 64:128), and row-disjoint matmuls overlap when
                  adjacent in the queue."""
                  lps = []
                  for hh in range(2):
                      lps.append(logits_ps.tile([P, QSH], F32, tag="lp",
                                                name="lp"))
                  for qb in range(QB):
                      for hh in range(2):
                          po = hh * D
                          nc.tensor.matmul(
                              lps[hh][:, qb * 512:(qb + 1) * 512],
                              kT[g][po:po + D, kt * P:(kt + 1) * P],
                              qT[g][po:po + D, qb * 512:(qb + 1) * 512],
                              start=True,
                              stop=True,
                          )
                  for hh in range(2):
                      at = attn_pool.tile([P, QSH], BF16, tag="attnT",
                                          name="at")
                      slot = (16 * g + kt) * 2 + hh
                      if use_dve(slot):
                          nc.vector.tensor_scalar(
                              at[:].bitcast(I16), lps[hh][:],
                              A_EXP, B_EXP,
                              mybir.AluOpType.mult, mybir.AluOpType.add,
                          )
                      else:
                          nc.scalar.activation(
                              out=at[:], in_=lps[hh][:],
                              func=mybir.ActivationFunctionType.Exp,
                          )
                      attn_tiles[(2 * g + hh, kt)] = at

              def consume(g, kt):
                  """AV (col-tiled pair) + denominator matmuls."""
                  if kt == 0:
                      cur_av[0] = av_ps.tile([P, QSH], F32, tag="av",
                                             name="av")
                      cur_den[0] = den_ps.tile([P, 512], F32, tag="den",
                                               name="den")
                      # keep never-written rows finite: normalize()'s
                      # reciprocal covers the whole bank.
                      nc.vector.memset(cur_den[0][:], 1.0)
                  av, den = cur_av[0], cur_den[0]
                  a_e = attn_tiles.pop((2 * g, kt))
                  a_o = attn_tiles.pop((2 * g + 1, kt))
                  st, sp = (kt == 0), (kt == KT - 1)
                  # AV matmuls alternate heads: the pair is col-tiled on
                  # the PE (cols 0:64 / 64:128), so adjacent col-disjoint
                  # matmuls overlap.  The four den matmuls go last, after
                  # all AV streaming, at four disjoint col positions, so
                  # they overlap each other.
                  for qb in range(QB):
                      qs = slice(qb * 512, (qb + 1) * 512)
                      nc.tensor.matmul(
                          av[0:D, qs], vt[kt][:, g * P:g * P + D],
                          a_e[:, qs], start=st, stop=sp,
                      )
                      nc.tensor.matmul(
                          av[D:2 * D, qs],
                          vt[kt][:, g * P + D:(g + 1) * P],
                          a_o[:, qs], start=st, stop=sp,
                      )
                  for qb in range(QB):
                      if _SKIP_DEN and not st:
                          break
                      # den rows: qb0 -> (e@0, o@32), qb1 -> (e@64, o@96)
                      # so each qb's pair is 32 apart for the K=33
                      # selection broadcast in normalize().
                      p0 = 64 * qb
                      nc.tensor.matmul(
                          den[p0:p0 + 1, :], ones128[:],
                          a_e[:, qb * 512:(qb + 1) * 512],
                          start=st, stop=(sp or _SKIP_DEN),
                          tile_position=(0, p0),
                      )
                      p1 = 64 * qb + 32
                      nc.tensor.matmul(
                          den[p1:p1 + 1, :], ones128[:],
                          a_o[:, qb * 512:(qb + 1) * 512],
                          start=st, stop=(sp or _SKIP_DEN),
                          tile_position=(0, p1),
                      )

              def normalize(g):
                  """reciprocal + broadcast + apply; writes aoT[g].

                  One DVE reciprocal covers the whole den bank (rows
                  0/32/64/96 are real; the rest was memset to 1.0).  Per
                  qb, ONE K=33 matmul against the selection matrix maps
                  recip rows {b, b+32} onto output partitions 0:64 /
                  64:128, and a single [128,512] tensor_tensor applies
                  both heads."""
                  av, den = cur_av[0], cur_den[0]
                  rf = small.tile([P, 512], F32, tag="rf", name="rf")
                  nc.vector.reciprocal_approx_fast(rf[:], den[:])
                  rr = small.tile([P, 512], F32R, tag="rr", name="rr")
                  nc.vector.tensor_copy(rr[:], rf[:])
                  for qb in range(QB):
                      b = 64 * qb
                      bc = proj_ps.tile([P, 512], F32, tag="proj",
                                        name="bc")
                      nc.tensor.matmul(
                          bc[:], sel[b:b + 33, :], rr[b:b + 33, :],
                          start=True, stop=True,
                          tile_position=(b, 0) if b else None,
                      )
                      bs = small.tile([P, 512], F32R, tag="bs", name="bs")
                      nc.vector.tensor_copy(bs[:], bc[:])
                      qs = slice(qb * 512, (qb + 1) * 512)
                      nc.vector.tensor_tensor(
                          out=aoT[g][:, qs],
                          in0=av[:, qs],
                          in1=bs[:],
                          op=mybir.AluOpType.mult,
                      )

              def outproj(qt):
                  # alternate PSUM banks (proj / recycled av) so the
                  # matmul group of qt+1 overlaps the eviction of qt
                  if qt % 2 == 0:
                      po = proj_ps.tile([P, 512], F32, tag="proj",
                                        name="ps_o")
                  else:
                      po = av_ps.tile([P, 512], F32, tag="av", name="ps_o2")
                  for g in range(CT):
                      nc.tensor.matmul(
                          po[:],
                          aoT[g][:, qt * P:(qt + 1) * P],
                          wp[:, g, :],
                          start=(g == 0),
                          stop=(g == CT - 1 and not with_bias),
                      )
                  if with_bias:
                      nc.tensor.matmul(po[:], onec[:], bp_r[:],
                                       start=False, stop=True)
                  o = outst.tile([P, C], F32, tag="outst", name="outst")
                  nc.vector.tensor_copy(o[:], po[:])
                  nc.sync.dma_start(out=out[qt * P:(qt + 1) * P, :], in_=o[:])

              # ---- unified 64-step pipeline: produce(s) then
              # consume(s-1); normalize(g) inline after consume(g, 15);
              # projection groups ride as per-step extras ----
              extras = {s: [] for s in range(64)}

              def put(s, fn, *a):
                  extras[s].append((fn, a))

              # pair 0 window: V projections land just-in-time; kT[0]
              # chunks prefetch one kb ahead; mt1 projections by step 16
              for kt in range(1, KT):
                  put(kt - 1, proj_v, kt)
              put(0, proj_kT, 0, 1)
              put(3, proj_kT, 0, 2)
              put(6, proj_kT, 0, 3)
              put(1, proj_qT, 1, 0)
              put(4, proj_qT, 1, 1)
              for kb in range(KB):
                  put(8 + 2 * kb, proj_kT, 1, kb)
              # pair 1 window: mt2 by step 32, mt3 by step 48
              put(16, proj_qT, 2, 0)
              put(18, proj_qT, 2, 1)
              for kb in range(KB):
                  put(20 + 2 * kb, proj_kT, 2, kb)
              put(28, proj_qT, 3, 0)
                  # mt3 K-projections stretch into the pair-2 window
              put(30, proj_qT, 3, 1)
              for kb in range(KB):
                  put(32 + 2 * kb, proj_kT, 3, kb)

              proj_qT(0, 0)
              proj_qT(0, 1)
              proj_kT(0, 0)
              proj_v(0)
              if _NO_EXTRAS:
                  # diagnostic: run every projection upfront; the steady
                  # state then carries no extra PE work (slow ramp).
                  moved = [t for s in sorted(extras) for t in extras[s]]
                  extras = {s: [] for s in range(64)}
                  for fn, a in moved:
                      fn(*a)
              steps = [(g, kt) for g in range(CT) for kt in range(KT)]
              if _PRODUCE_ONLY:
                  # cadence probe: logits+exp stream only (WRONG results).
                  for kb in range(1, KB):
                      proj_kT(0, kb)
                  for s in range(len(steps)):
                      produce(0, s % KT)
                      attn_tiles.clear()
                  for g in range(CT):
                      nc.vector.tensor_copy(aoT[g][:], qT[0][:])
              else:
                  LAG = 2  # consume deps are 2 steps old: no PE waits
                  for s, (g, kt) in enumerate(steps):
                      produce(g, kt)
                      for fn, a in extras[s]:
                          fn(*a)
                      if s >= LAG:
                          pg, pkt = steps[s - LAG]
                          consume(pg, pkt)
                          if pkt == KT - 1:
                              normalize(pg)
                  for s in range(len(steps) - LAG, len(steps)):
                      pg, pkt = steps[s]
                      consume(pg, pkt)
                      if pkt == KT - 1:
                          normalize(pg)

              # ---- output projection tail ----
              for qt in range(QT):
                  outproj(qt)

    nc.compile()
    return nc


_CACHE: dict = {}


def get_nc(with_bias: bool):
    key = ("nc", with_bias)
    if key not in _CACHE:
        _CACHE[key] = build_nc(with_bias)
    return _CACHE[key]


def make_in_maps(inputs: dict) -> tuple[list[dict], bool]:
    x = np.asarray(inputs["x"], dtype=np.float32)
    mlp = np.asarray(inputs["mlp_out"], dtype=np.float32)
    Wq = np.asarray(inputs["Wq"], dtype=np.float32)
    Wk = np.asarray(inputs["Wk"], dtype=np.float32)
    Wv = np.asarray(inputs["Wv"], dtype=np.float32)
    Wp = np.asarray(inputs["Wp"], dtype=np.float32)
    bq = np.asarray(inputs["bq"], dtype=np.float32)
    bk = np.asarray(inputs["bk"], dtype=np.float32)
    bv = np.asarray(inputs["bv"], dtype=np.float32)
    bp = np.asarray(inputs["bp"], dtype=np.float32)

    with_bias = bool(np.any(bq) or np.any(bk) or np.any(bv) or np.any(bp))

    wqT = np.ascontiguousarray(Wq.T)  # [c, dh]
    wkT = np.ascontiguousarray(Wk.T)
    wvT = np.ascontiguousarray(Wv.T)
    wpT = np.ascontiguousarray(Wp.T)  # [dh, co]

    in_maps = []
    for c in range(NCORES):
        b, half = c // 2, c % 2
        xs = np.ascontiguousarray(x[b, half * QSH:(half + 1) * QSH, :].T)
        ms = np.ascontiguousarray(mlp[b].T)
        m = {
            "xT": xs, "mlpT": ms,
            "wqT": wqT, "wkT": wkT, "wvT": wvT, "wpT": wpT,
        }
        if with_bias:
            m["bq"] = bq.reshape(1, C)
            m["bk"] = bk.reshape(1, C)
            m["bv"] = bv.reshape(1, C)
            m["bp"] = bp.reshape(1, C)
        in_maps.append(m)
    return in_maps, with_bias


def kernel(**inputs) -> np.ndarray:
    in_maps, with_bias = make_in_maps(inputs)
    nc = get_nc(with_bias)
    res = run_bass_kernel_spmd(nc, in_maps, list(range(NCORES)))
    full = np.empty((B, N, C), dtype=np.float32)
    for c in range(NCORES):
        b, half = c // 2, c % 2
        full[b, half * QSH:(half + 1) * QSH, :] = res.results[c]["out"]
    return full



# revision 46
# speedup vs baseline: 1.6199x; 1.1745x over previous
"""Cross-attention kernel for Trainium2, SPMD across 8 NeuronCores.

Problem shapes (hardcoded): x [4, 2048, 512], mlp_out [4, 2048, 512],
Wq/Wk/Wv/Wp [512, 512], biases [512]. 8 heads x 64 head-dim.

Sharding: core c handles batch b = c//2 and query rows
[(c%2)*1024 : (c%2+1)*1024).  K/V work is duplicated across the two
cores of a batch pair; in exchange no collective is needed.

Design notes (v2):
  - The scalar engine's exp stream (128 x [128,1024] ~ 133us) is the
    hard floor; everything else is shaped to fit underneath it.
  - logits are computed transposed ([k, q]) per head with the head
    PAIR row-tiled on the PE array (contraction=64 at array rows
    0:64 / 64:128, concurrent), so a pair's two [128,1024] logit
    tiles cost ~the cycles of one.
  - AV makes V the stationary operand (64-col LDWEIGHTS instead of
    one 128-col load per tiny matmul) and the bf16 attn tile the
    moving operand; the two heads of a pair col-tile the array
    (outputs at PSUM partitions 0:64 / 64:128, concurrent).  The AV
    output is [dh, q] - already transposed for the output
    projection, so no PE transposes at all.
  - softmax denominators come from ones-vector matmuls col-tiled
    4-way (partitions 0/32/64/96 of one PSUM bank); the reciprocal
    row is broadcast across 64 partitions by a rank-1 matmul and
    applied as one [128,512] DVE multiply during the AV eviction.
  - attention is software-pipelined per (pair, kt): exp for step s
    is emitted before AV for step s-1, so the scalar engine never
    waits on the PE and attn tiles free after one step.
  - PSUM is exactly 8 banks: logits 2x[128,1024] (4) + AV [128,1024]
    (2) + denominators [128,512] (1) + one projection bank.  The
    projection bank is single-buffered; its eviction latency hides
    behind interleaved attention matmuls, and the mt>=2 Q/K
    projection groups are deferred into the pair-1 attention stream
    (their results aren't needed until pairs 2/3).
"""

import os

import numpy as np

import concourse.bass as bass
import concourse.tile as tile
from concourse import bacc, mybir
from concourse.bass_utils import run_bass_kernel_spmd

# timing-only ablation knobs (default off; results are WRONG when set)
_SKIP_DEN = os.environ.get("K_SKIP_DEN") == "1"
_SKIP_AVODD = os.environ.get("K_SKIP_AVODD") == "1"
_PRODUCE_ONLY = os.environ.get("K_PRODUCE_ONLY") == "1"
_NO_EXTRAS = os.environ.get("K_NO_EXTRAS") == "1"
_LP_BUFS = int(os.environ.get("K_LP_BUFS", "2"))
# exp engine schedule:
#   "quad"  (default): logits live in four 1-bank [128,512] psum tiles
#           per step; each gets its own exp (scalar Exp or DVE
#           Schraudolph bit-exp), so a logits bank is freed by a short
#           ~700ns op instead of a 1147ns full-tile exp -- the
#           produce->exp->reuse chain shortens accordingly.  K_QUAD_N
#           of every 256 quarter-exps go to the DVE (Bresenham).
#   "pair": whole-tile exps; even head ScalarE, odd head DVE.
#   "mixN": whole-tile exps, N of 128 on DVE.
# The DVE bit-exp (tensor_scalar i16 = round(x*2^7/ln2 + B)) writes
# int16 bits that ARE bf16(exp(x)) up to a ~3% sawtooth which largely
# cancels in softmax.
_EXP_MODE = os.environ.get("K_EXP_MODE", "quad")
_QUAD_N = int(os.environ.get("K_QUAD_N", "100"))
# alternate projection-eviction copies between DVE and ScalarE so they
# don't all queue ahead of the DVE's exp work (head-of-line blocking)
_EVICT_MIX = os.environ.get("K_EVICT_MIX", "0") == "1"

B = 4
N = 2048          # both query and key/value sequence length
C = 512           # model dim
H = 8
D = C // H        # 64
NCORES = 8
QSH = N // 2      # query rows per core (1024)

F32 = mybir.dt.float32
F32R = mybir.dt.float32r
BF16 = mybir.dt.bfloat16
I16 = mybir.dt.int16

import math
# Schraudolph bit-exp in bf16-bit domain: i16 = round(x * 2^7/ln2 +
# (127*2^7 - C)); the int16 bit pattern IS bf16(exp(x)) up to a ~3%
# sawtooth (C centers it).  The constant offset cancels in softmax.
A_EXP = float(np.float32(2.0**7 / math.log(2.0)))
B_EXP = float(np.float32(127.0 * 2.0**7 - 5.5))

P = 128
CT = C // P       # 4 tiles along any model-dim axis (also: head pairs)
QT = QSH // P     # 8 query tiles
KT = N // P       # 16 key tiles
QB = QSH // 512   # 2 query blocks of 512 (fp32-class moving-dim limit)
KB = N // 512     # 4


def build_nc(with_bias: bool, reps: int = 1):
    nc = bacc.Bacc("TRN2", target_bir_lowering=False, debug=False)

    xT = nc.dram_tensor("xT", [C, QSH], F32R, kind="ExternalInput")
    mlpT = nc.dram_tensor("mlpT", [C, N], F32R, kind="ExternalInput")
    wqT = nc.dram_tensor("wqT", [C, C], F32R, kind="ExternalInput")
    wkT = nc.dram_tensor("wkT", [C, C], F32R, kind="ExternalInput")
    wvT = nc.dram_tensor("wvT", [C, C], F32R, kind="ExternalInput")
    wpT = nc.dram_tensor("wpT", [C, C], F32R, kind="ExternalInput")
    if with_bias:
        bq = nc.dram_tensor("bq", [1, C], F32, kind="ExternalInput")
        bk = nc.dram_tensor("bk", [1, C], F32, kind="ExternalInput")
        bv = nc.dram_tensor("bv", [1, C], F32, kind="ExternalInput")
        bp = nc.dram_tensor("bp", [1, C], F32, kind="ExternalInput")
    out = nc.dram_tensor("out", [QSH, C], F32, kind="ExternalOutput")

    with tile.TileContext(nc) as tc:
        from contextlib import ExitStack

        with ExitStack() as ctx:
            const = ctx.enter_context(tc.tile_pool(name="const", bufs=1))
            wq_pool = ctx.enter_context(tc.tile_pool(name="wq", bufs=1))
            wk_pool = ctx.enter_context(tc.tile_pool(name="wk", bufs=1))
            wv_pool = ctx.enter_context(tc.tile_pool(name="wv", bufs=1))
            wp_pool = ctx.enter_context(tc.tile_pool(name="wp", bufs=1))
            xin_pool = ctx.enter_context(tc.tile_pool(name="xin", bufs=2))
            min_pool = ctx.enter_context(tc.tile_pool(name="min", bufs=4))
            qt_pool = ctx.enter_context(tc.tile_pool(name="qTp", bufs=1))
            kt_pool = ctx.enter_context(tc.tile_pool(name="kTp", bufs=1))
            v_pool = ctx.enter_context(tc.tile_pool(name="vp", bufs=1))
            aoT_pool = ctx.enter_context(tc.tile_pool(name="aoT", bufs=1))
            attn_pool = ctx.enter_context(tc.tile_pool(name="attnT", bufs=10))
            small = ctx.enter_context(tc.tile_pool(name="small", bufs=2))
            outst = ctx.enter_context(tc.tile_pool(name="outst", bufs=2))
            logits_ps = ctx.enter_context(
                tc.tile_pool(
                    name="logits_ps",
                    bufs=(5 if _EXP_MODE == "quad" else _LP_BUFS),
                    space="PSUM",
                )
            )
            proj_ps = logits_ps
            av_ps = ctx.enter_context(
                tc.tile_pool(name="av_ps", bufs=1, space="PSUM")
            )
            den_ps = ctx.enter_context(
                tc.tile_pool(name="den_ps", bufs=1, space="PSUM")
            )

            # ---- constants ----
            onesr_f = const.tile([1, P], F32)
            nc.vector.memset(onesr_f[:], 1.0)
            onesr = const.tile([1, P], F32R)
            nc.vector.tensor_copy(onesr[:], onesr_f[:])
            ones128 = const.tile([P, 1], BF16)
            nc.vector.memset(ones128[:], 1.0)
            # selection matrix for the normalize broadcast: one K=33
            # matmul maps recip rows {b, b+32} onto output partitions
            # 0:64 / 64:128 (b = 0 for qb0, 64 for qb1).
            sel_f = const.tile([P, P], F32)
            nc.vector.memset(sel_f[:], 0.0)
            for b in (0, 64):
                nc.vector.memset(sel_f[b:b + 1, 0:64], 1.0)
                nc.vector.memset(sel_f[b + 32:b + 33, 64:128], 1.0)
            sel = const.tile([P, P], F32R)
            nc.vector.tensor_copy(sel[:], sel_f[:])

            for _rep in range(reps):

              if with_bias:
                  def load_row_f32r(dram_row, nm):
                      f = const.tile([1, C], F32, name=f"{nm}_f")
                      nc.sync.dma_start(out=f[:], in_=dram_row[:])
                      r = const.tile([1, C], F32R, name=f"{nm}_r")
                      nc.vector.tensor_copy(r[:], f[:])
                      return r

                  bq_r = load_row_f32r(bq, "bq")
                  bk_r = load_row_f32r(bk, "bk")
                  bv_r = load_row_f32r(bv, "bv")
                  bp_r = load_row_f32r(bp, "bp")
                  ones_f = const.tile([1, 512], F32)
                  nc.vector.memset(ones_f[:], 1.0)
                  ones_r = const.tile([1, 512], F32R)
                  nc.vector.tensor_copy(ones_r[:], ones_f[:])
                  onec = const.tile([1, P], F32R)
                  nc.vector.tensor_copy(onec[:], ones_r[:, 0:P])

              # ---- input DMAs.  Weight tiles are [128, 4, 512]: wq[p,cc,n]
              # = wqT[cc*128+p, n].  wq/wk arrive split so the mt=0 slice
              # (all pair-0 needs) lands as early as possible. ----
              def w_tile(pool, nm):
                  return pool.tile([P, CT, C], F32R, tag=nm, name=nm)

              def dma_w(t, dram, n0, n1):
                  nc.sync.dma_start(
                      out=t[:, :, n0:n1],
                      in_=dram[:, n0:n1].rearrange("(c p) n -> p c n", p=P),
                  )

              wq = w_tile(wq_pool, "wq")
              dma_w(wq, wqT, 0, P)
              xblks = []
              for qb in range(QB):
                  t = xin_pool.tile([P, CT, 512], F32R, tag="xin",
                                    name=f"xb{qb}")
                  nc.sync.dma_start(
                      out=t[:],
                      in_=xT[:, qb * 512:(qb + 1) * 512].rearrange(
                          "(c p) n -> p c n", p=P),
                  )
                  xblks.append(t)
              wk = w_tile(wk_pool, "wk")
              dma_w(wk, wkT, 0, P)
              mblks = []
              for kb in range(KB):
                  t = min_pool.tile([P, CT, 512], F32R, tag="min",
                                    name=f"mb{kb}")
                  if kb == 0:
                      nc.sync.dma_start(
                          out=t[:],
                          in_=mlpT[:, 0:512].rearrange("(c p) n -> p c n",
                                                       p=P),
                      )
                  mblks.append(t)
              wv = w_tile(wv_pool, "wv")
              dma_w(wv, wvT, 0, C)
              dma_w(wq, wqT, P, C)
              dma_w(wk, wkT, P, C)
              for kb in range(1, KB):
                  nc.sync.dma_start(
                      out=mblks[kb][:],
                      in_=mlpT[:, kb * 512:(kb + 1) * 512].rearrange(
                          "(c p) n -> p c n", p=P),
                  )
              wp = w_tile(wp_pool, "wp")
              dma_w(wp, wpT, 0, C)

              qT = [qt_pool.tile([P, QSH], F32R, tag=f"qT{i}", name=f"qT{i}")
                    for i in range(CT)]
              kT = [kt_pool.tile([P, N], F32R, tag=f"kT{i}", name=f"kT{i}")
                    for i in range(CT)]
              # v tiles: [k-part, dh] per kt; lhsT slices are 64-col head dims
              vt = [v_pool.tile([P, C], BF16, tag=f"v{i}", name=f"v{i}")
                    for i in range(KT)]
              # attn-out transposed: per pair g, [128 dh, 1024 q]
              aoT = [aoT_pool.tile([P, QSH], F32R, tag=f"aoT{g}",
                                   name=f"aoT{g}") for g in range(CT)]

              def proj_qT(mt, qb):
                  ps = proj_ps.tile([P, 512], F32, tag="lp", name="ps_q")
                  for cc in range(CT):
                      nc.tensor.matmul(
                          ps[:],
                          wq[:, cc, mt * P:(mt + 1) * P],
                          xblks[qb][:, cc, :],
                          start=(cc == 0),
                          stop=(cc == CT - 1 and not with_bias),
                      )
                  if with_bias:
                      nc.tensor.matmul(
                          ps[:], bq_r[:, mt * P:(mt + 1) * P], ones_r[:],
                          start=False, stop=True,
                      )
                  nc.vector.tensor_copy(qT[mt][:, qb * 512:(qb + 1) * 512],
                                        ps[:])

              def proj_kT(mt, kb):
                  ps = proj_ps.tile([P, 512], F32, tag="lp", name="ps_k")
                  for cc in range(CT):
                      nc.tensor.matmul(
                          ps[:],
                          wk[:, cc, mt * P:(mt + 1) * P],
                          mblks[kb][:, cc, :],
                          start=(cc == 0),
                          stop=(cc == CT - 1 and not with_bias),
                      )
                  if with_bias:
                      nc.tensor.matmul(
                          ps[:], bk_r[:, mt * P:(mt + 1) * P], ones_r[:],
                          start=False, stop=True,
                      )
                  nc.vector.tensor_copy(kT[mt][:, kb * 512:(kb + 1) * 512],
                                        ps[:])

              def proj_v(kt):
                  kb, lo = kt // 4, (kt % 4) * P
                  ps = proj_ps.tile([P, 512], F32, tag="lp", name="ps_v")
                  for cc in range(CT):
                      nc.tensor.matmul(
                          ps[:],
                          mblks[kb][:, cc, lo:lo + P],
                          wv[:, cc, :],
                          start=(cc == 0),
                          stop=(cc == CT - 1 and not with_bias),
                      )
                  if with_bias:
                      nc.tensor.matmul(ps[:], onec[:], bv_r[:],
                                       start=False, stop=True)
                  if _EVICT_MIX:
                      # bf16 eviction can go through ScalarE, keeping
                      # the DVE queue clear for its exp stream
                      nc.scalar.copy(vt[kt][:], ps[:])
                  else:
                      nc.vector.tensor_copy(vt[kt][:], ps[:])

              # ---- attention pipeline ----
              cur_av = [None]    # [128,1024] psum: 0:64 even / 64:128 odd
              cur_den = [None]   # [128,512] psum: denoms at 0/32/64/96
              attn_tiles = {}    # (h, kt) -> sbuf tile [128, 1024]

              if _EXP_MODE.startswith("mix"):
                  _n_mix = int(_EXP_MODE[3:])

                  def use_dve(slot):
                      return (((slot + 1) * _n_mix) // 128
                              > (slot * _n_mix) // 128)
              else:
                  def use_dve(slot):
                      return slot % 2 == 1   # odd head -> DVE

              def use_dve_q(slot4):
                  return (((slot4 + 1) * _QUAD_N) // 256
                          > (slot4 * _QUAD_N) // 256)

              def exp_op(dve, out_ap, in_ap):
                  if dve:
                      nc.vector.tensor_scalar(
                          out_ap.bitcast(I16), in_ap,
                          A_EXP, B_EXP,
                          mybir.AluOpType.mult, mybir.AluOpType.add,
                      )
                  else:
                      nc.scalar.activation(
                          out=out_ap, in_=in_ap,
                          func=mybir.ActivationFunctionType.Exp,
                      )

              def produce(g, kt):
                  """logits + exp for step (g, kt).

                  quad mode: each (head, qb) matmul gets its own 1-bank
                  psum tile and its own exp, so banks recycle at the
                  cadence of a half-width exp."""
                  if _EXP_MODE == "quad":
                      ats = []
                      for hh in range(2):
                          at = attn_pool.tile([P, QSH], BF16, tag="attnT",
                                              name="at")
                          ats.append(at)
                          attn_tiles[(2 * g + hh, kt)] = at
                      for hh in range(2):
                          po = hh * D
                          for qb in range(QB):
                              lp = logits_ps.tile([P, 512], F32, tag="lp",
                                                  name="lp")
                              nc.tensor.matmul(
                                  lp[:],
                                  kT[g][po:po + D, kt * P:(kt + 1) * P],
                                  qT[g][po:po + D,
                                        qb * 512:(qb + 1) * 512],
                                  start=True,
                                  stop=True,
                              )
                              slot4 = (16 * g + kt) * 4 + 2 * hh + qb
                              exp_op(use_dve_q(slot4),
                                     ats[hh][:, qb * 512:(qb + 1) * 512],
                                     lp[:])
                      return
                  lps = []
                  for hh in range(2):
                      lps.append(logits_ps.tile([P, QSH], F32, tag="lp",
                                                name="lp"))
                  for hh in range(2):
                      po = hh * D
                      for qb in range(QB):
                          nc.tensor.matmul(
                              lps[hh][:, qb * 512:(qb + 1) * 512],
                              kT[g][po:po + D, kt * P:(kt + 1) * P],
                              qT[g][po:po + D, qb * 512:(qb + 1) * 512],
                              start=True,
                              stop=True,
                          )
                  for hh in range(2):
                      at = attn_pool.tile([P, QSH], BF16, tag="attnT",
                                          name="at")
                      slot = (16 * g + kt) * 2 + hh
                      exp_op(use_dve(slot), at[:], lps[hh][:])
                      attn_tiles[(2 * g + hh, kt)] = at

              def consume(g, kt):
                  """AV (col-tiled pair) + denominator matmuls."""
                  if kt == 0:
                      cur_av[0] = av_ps.tile([P, QSH], F32, tag="av",
                                             name="av")
                      cur_den[0] = den_ps.tile([P, 512], F32, tag="den",
                                               name="den")
                      # keep never-written rows finite: normalize()'s
                      # reciprocal covers the whole bank.
                      nc.vector.memset(cur_den[0][:], 1.0)
                  av, den = cur_av[0], cur_den[0]
                  a_e = attn_tiles.pop((2 * g, kt))
                  a_o = attn_tiles.pop((2 * g + 1, kt))
                  st, sp = (kt == 0), (kt == KT - 1)
                  # head-major AV order: the two qb matmuls of a head
                  # share their stationary (one weight load) and target
                  # adjacent banks back-to-back.
                  for hh, a_t in ((0, a_e), (1, a_o)):
                      vs = vt[kt][:, g * P + hh * D:g * P + (hh + 1) * D]
                      for qb in range(QB):
                          qs = slice(qb * 512, (qb + 1) * 512)
                          nc.tensor.matmul(
                              av[hh * D:(hh + 1) * D, qs], vs,
                              a_t[:, qs], start=st, stop=sp,
                          )
                  for qb in range(QB):
                      if _SKIP_DEN and not st:
                          break
                      # den rows: qb0 -> (e@0, o@32), qb1 -> (e@64, o@96)
                      # so each qb's pair is 32 apart for the K=33
                      # selection broadcast in normalize().
                      p0 = 64 * qb
                      nc.tensor.matmul(
                          den[p0:p0 + 1, :], ones128[:],
                          a_e[:, qb * 512:(qb + 1) * 512],
                          start=st, stop=(sp or _SKIP_DEN),
                          tile_position=(0, p0),
                      )
                      p1 = 64 * qb + 32
                      nc.tensor.matmul(
                          den[p1:p1 + 1, :], ones128[:],
                          a_o[:, qb * 512:(qb + 1) * 512],
                          start=st, stop=(sp or _SKIP_DEN),
                          tile_position=(0, p1),
                      )

              def normalize(g):
                  """reciprocal + broadcast + apply; writes aoT[g].

                  One DVE reciprocal covers the whole den bank (rows
                  0/32/64/96 are real; the rest was memset to 1.0).  Per
                  qb, ONE K=33 matmul against the selection matrix maps
                  recip rows {b, b+32} onto output partitions 0:64 /
                  64:128, and a single [128,512] tensor_tensor applies
                  both heads."""
                  av, den = cur_av[0], cur_den[0]
                  rf = small.tile([P, 512], F32, tag="rf", name="rf")
                  nc.vector.reciprocal_approx_fast(rf[:], den[:])
                  rr = small.tile([P, 512], F32R, tag="rr", name="rr")
                  nc.vector.tensor_copy(rr[:], rf[:])
                  for qb in range(QB):
                      b = 64 * qb
                      bc = proj_ps.tile([P, 512], F32, tag="lp", name="bc")
                      nc.tensor.matmul(
                          bc[:], sel[b:b + 33, :], rr[b:b + 33, :],
                          start=True, stop=True,
                          tile_position=(b, 0) if b else None,
                      )
                      bs = small.tile([P, 512], F32R, tag="bs", name="bs")
                      nc.vector.tensor_copy(bs[:], bc[:])
                      qs = slice(qb * 512, (qb + 1) * 512)
                      nc.vector.tensor_tensor(
                          out=aoT[g][:, qs],
                          in0=av[:, qs],
                          in1=bs[:],
                          op=mybir.AluOpType.mult,
                      )

              def outproj(qt):
                  # alternate PSUM banks (proj / recycled av) so the
                  # matmul group of qt+1 overlaps the eviction of qt
                  if qt % 2 == 0:
                      po = proj_ps.tile([P, 512], F32, tag="lp", name="ps_o")
                  else:
                      po = av_ps.tile([P, 512], F32, tag="av", name="ps_o2")
                  for g in range(CT):
                      nc.tensor.matmul(
                          po[:],
                          aoT[g][:, qt * P:(qt + 1) * P],
                          wp[:, g, :],
                          start=(g == 0),
                          stop=(g == CT - 1 and not with_bias),
                      )
                  if with_bias:
                      nc.tensor.matmul(po[:], onec[:], bp_r[:],
                                       start=False, stop=True)
                  o = outst.tile([P, C], F32, tag="outst", name="outst")
                  if _EVICT_MIX:
                      nc.scalar.copy(o[:], po[:])
                  else:
                      nc.vector.tensor_copy(o[:], po[:])
                  nc.sync.dma_start(out=out[qt * P:(qt + 1) * P, :], in_=o[:])

              # ---- unified 64-step pipeline: produce(s) then
              # consume(s-1); normalize(g) inline after consume(g, 15);
              # projection groups ride as per-step extras ----
              extras = {s: [] for s in range(64)}

              def put(s, fn, *a):
                  extras[s].append((fn, a))

              # pair 0 window: V projections land just-in-time; kT[0]
              # chunks prefetch one kb ahead; mt1 projections by step 16
              for kt in range(1, KT):
                  put(kt - 1, proj_v, kt)
              put(0, proj_kT, 0, 1)
              put(3, proj_kT, 0, 2)
              put(6, proj_kT, 0, 3)
              put(1, proj_qT, 1, 0)
              put(4, proj_qT, 1, 1)
              for kb in range(KB):
                  put(8 + 2 * kb, proj_kT, 1, kb)
              # pair 1 window: mt2 by step 32, mt3 by step 48
              put(16, proj_qT, 2, 0)
              put(18, proj_qT, 2, 1)
              for kb in range(KB):
                  put(20 + 2 * kb, proj_kT, 2, kb)
              put(28, proj_qT, 3, 0)
                  # mt3 K-projections stretch into the pair-2 window
              put(30, proj_qT, 3, 1)
              for kb in range(KB):
                  put(32 + 2 * kb, proj_kT, 3, kb)

              proj_qT(0, 0)
              proj_qT(0, 1)
              proj_kT(0, 0)
              proj_v(0)
              if _NO_EXTRAS:
                  # diagnostic: run every projection upfront; the steady
                  # state then carries no extra PE work (slow ramp).
                  moved = [t for s in sorted(extras) for t in extras[s]]
                  extras = {s: [] for s in range(64)}
                  for fn, a in moved:
                      fn(*a)
              steps = [(g, kt) for g in range(CT) for kt in range(KT)]
              if _PRODUCE_ONLY:
                  # cadence probe: logits+exp stream only (WRONG results).
                  for kb in range(1, KB):
                      proj_kT(0, kb)
                  for s in range(len(steps)):
                      produce(0, s % KT)
                      attn_tiles.clear()
                  for g in range(CT):
                      nc.vector.tensor_copy(aoT[g][:], qT[0][:])
              else:
                  LAG = 2  # consume deps are 2 steps old: no PE waits
                  for s, (g, kt) in enumerate(steps):
                      produce(g, kt)
                      for fn, a in extras[s]:
                          fn(*a)
                      if s >= LAG:
                          pg, pkt = steps[s - LAG]
                          consume(pg, pkt)
                          if pkt == KT - 1:
                              normalize(pg)
                  for s in range(len(steps) - LAG, len(steps)):
                      pg, pkt = steps[s]
                      consume(pg, pkt)
                      if pkt == KT - 1:
                          normalize(pg)

              # ---- output projection tail ----
              for qt in range(QT):
                  outproj(qt)

    nc.compile()
    return nc


_CACHE: dict = {}


def get_nc(with_bias: bool):
    key = ("nc", with_bias)
    if key not in _CACHE:
        _CACHE[key] = build_nc(with_bias)
    return _CACHE[key]


def make_in_maps(inputs: dict) -> tuple[list[dict], bool]:
    x = np.asarray(inputs["x"], dtype=np.float32)
    mlp = np.asarray(inputs["mlp_out"], dtype=np.float32)
    Wq = np.asarray(inputs["Wq"], dtype=np.float32)
    Wk = np.asarray(inputs["Wk"], dtype=np.float32)
    Wv = np.asarray(inputs["Wv"], dtype=np.float32)
    Wp = np.asarray(inputs["Wp"], dtype=np.float32)
    bq = np.asarray(inputs["bq"], dtype=np.float32)
    bk = np.asarray(inputs["bk"], dtype=np.float32)
    bv = np.asarray(inputs["bv"], dtype=np.float32)
    bp = np.asarray(inputs["bp"], dtype=np.float32)

    with_bias = bool(np.any(bq) or np.any(bk) or np.any(bv) or np.any(bp))

    wqT = np.ascontiguousarray(Wq.T)  # [c, dh]
    wkT = np.ascontiguousarray(Wk.T)
    wvT = np.ascontiguousarray(Wv.T)
    wpT = np.ascontiguousarray(Wp.T)  # [dh, co]

    in_maps = []
    for c in range(NCORES):
        b, half = c // 2, c % 2
        xs = np.ascontiguousarray(x[b, half * QSH:(half + 1) * QSH, :].T)
        ms = np.ascontiguousarray(mlp[b].T)
        m = {
            "xT": xs, "mlpT": ms,
            "wqT": wqT, "wkT": wkT, "wvT": wvT, "wpT": wpT,
        }
        if with_bias:
            m["bq"] = bq.reshape(1, C)
            m["bk"] = bk.reshape(1, C)
            m["bv"] = bv.reshape(1, C)
            m["bp"] = bp.reshape(1, C)
        in_maps.append(m)
    return in_maps, with_bias


def kernel(**inputs) -> np.ndarray:
    in_maps, with_bias = make_in_maps(inputs)
    nc = get_nc(with_bias)
    res = run_bass_kernel_spmd(nc, in_maps, list(range(NCORES)))
    full = np.empty((B, N, C), dtype=np.float32)
    for c in range(NCORES):
        b, half = c // 2, c % 2
        full[b, half * QSH:(half + 1) * QSH, :] = res.results[c]["out"]
    return full

